# revision 1
# baseline (speedup 1.0000x reference)
"""TRN2 Bass kernel for nn_Actor (retrieval_knn).

Data-parallel over batch across 8 NeuronCores (8192 rows/core).

The execution environment pays a large fixed cost per *instruction*
(measured ~10us/instruction regardless of operand size or engine
parallelism), so this kernel minimizes dynamic instruction count:

- ONE ap_gather builds the MLP input directly: channels grouped so each
  Q7 core (16 channels sharing an index stream) serves one
  (table, batch-half) pair -> x64 [64, 4096] with worker/project
  features of both batch halves stacked on partitions.
- MLP layer 1 as 8 matmuls with a [64, 80] block lhsT computing two
  batch-halves at once ([80, 512] PSUM outputs), 2 fused ReLUs.
- h split into bf16 pair (h1, h2 = h - h1) with 2 elementwise ops, then
  6 DMAs assemble the 122-row stacked lhs (W2 absorbed into the table:
  scores = h1G1 + h1G2 + h2G1 + c1 + c2, fp32-grade accuracy).
- Scores + argmax: per 128-row tile, 5 matmuls (PSUM-bank-sized) with
  lhsT taken directly from the stacked buffer (no staging copy), then
  DVE max8 + max_index on the fp32 PSUM scores: exact argmax.
"""
import sys
sys.path.insert(0, '/opt/trn_rl_repo')
import numpy as np
import ml_dtypes

B = 65536
NCORES = 8
BC = B // NCORES            # 8192
NW, NPTAB, EMB = 1807, 2490, 10
NPROJ = NPTAB - 1           # 2489
HID = 40
NTILES = BC // 128          # 64
HALF = BC // 2              # 4096

_cache = {}


def _bf16(x):
    return np.asarray(x, np.float32).astype(ml_dtypes.bfloat16)


def _build(L=1, hwloop=True, tk=1, sr=True, cpe="vector"):
    from concourse import bacc, mybir, bass
    from concourse.tile import TileContext
    import concourse.mybir as mb
    dt = mybir.dt
    AF = mb.ActivationFunctionType
    nc = bacc.Bacc("TRN2", target_bir_lowering=False, debug=False, num_devices=NCORES)

    xidx = nc.dram_tensor("xidx", [64, HALF // 16], dt.int16, kind="ExternalInput")
    xtab = nc.dram_tensor("xtab", [64, NPTAB], dt.float32, kind="ExternalInput")
    w1s = nc.dram_tensor("w1s", [64, 80], dt.float32, kind="ExternalInput")
    b1e = nc.dram_tensor("b1e", [80, 1], dt.float32, kind="ExternalInput")
    tstk = nc.dram_tensor("tstk", [122, NPROJ], dt.bfloat16, kind="ExternalInput")
    out_ext = nc.dram_tensor("out", [128, NTILES * 8], dt.uint32, kind="ExternalOutput")

    with TileContext(nc) as tc:
        with tc.tile_pool(name="const", bufs=1) as cp, \
             tc.tile_pool(name="work", bufs=1) as wp, \
             tc.tile_pool(name="sc", bufs=1, space="PSUM") as scp:
            t_xtab = cp.tile([64, NPTAB], dt.float32)
            t_xidx = cp.tile([64, HALF // 16], dt.int16)
            t_w1s = cp.tile([64, 80], dt.float32)
            t_b1 = cp.tile([80, 1], dt.float32)
            t_tstk = cp.tile([122, NPROJ], dt.bfloat16)
            nc.sync.dma_start(out=t_xtab, in_=xtab.ap())
            nc.sync.dma_start(out=t_xidx, in_=xidx.ap())
            nc.sync.dma_start(out=t_w1s, in_=w1s.ap())
            nc.sync.dma_start(out=t_b1, in_=b1e.ap())
            nc.sync.dma_start(out=t_tstk, in_=tstk.ap())

            x64 = wp.tile([64, HALF], dt.float32)
            h80 = wp.tile([80, BC // 2], dt.float32)
            hs1 = wp.tile([80, BC // 2], dt.bfloat16)
            hs2 = wp.tile([80, BC // 2], dt.bfloat16)
            hstack = wp.tile([122, BC], dt.bfloat16)
            onesrow = wp.tile([2, BC], dt.bfloat16)
            outbuf = wp.tile([128, NTILES * 8], dt.uint32)
            wstage = wp.tile([122, 128 * tk], dt.bfloat16)
            m8 = wp.tile([128, 8], dt.float32)
            nc.vector.memset(onesrow, 1.0)
            nc.sync.dma_start(out=hstack[120:122, :], in_=onesrow)
            ps = scp.tile([128, NPROJ], dt.float32)

            for _ in range(L):
                nc.gpsimd.ap_gather(out_ap=x64, in_ap=t_xtab, idxs_ap=t_xidx,
                                    channels=64, num_elems=NPTAB, d=1,
                                    num_idxs=HALF)
                # MLP layer 1: [64,80] block lhsT -> [80,512] PSUM chunks,
                # two fused 4-bank ReLUs (fully unrolled: only 10 instructions)
                for grp in range(2):
                    for k in range(4):
                        c = grp * 4 + k
                        nc.tensor.matmul(ps[0:80, k * 512:(k + 1) * 512],
                                         lhsT=t_w1s[:, :],
                                         rhs=x64[:, c * 512:(c + 1) * 512],
                                         start=True, stop=True)
                    nc.scalar.activation(h80[:, grp * 2048:(grp + 1) * 2048],
                                         ps[0:80, 0:2048],
                                         mb.ActivationFunctionType.Relu,
                                         bias=t_b1)
                # bf16 split and 122-row lhs assembly
                nc.vector.tensor_copy(hs1[:, :], h80[:, :])
                nc.gpsimd.tensor_sub(hs2[:, :], h80[:, :], hs1[:, :])
                nc.sync.dma_start(out=hstack[0:40, 0:HALF], in_=hs1[0:40, :])
                nc.scalar.dma_start(out=hstack[0:40, HALF:BC], in_=hs1[40:80, :])
                nc.sync.dma_start(out=hstack[64:104, 0:HALF], in_=hs2[0:40, :])
                nc.scalar.dma_start(out=hstack[64:104, HALF:BC], in_=hs2[40:80, :])
                nc.sync.dma_start(out=hstack[40:64, :], in_=hstack[0:24, :])
                nc.scalar.dma_start(out=hstack[104:120, :], in_=hstack[24:40, :])

                # scores + argmax: tk tiles per loop iteration share ONE
                # wstage group copy (ldweights needs static lhsT addresses,
                # which the static slices of the group buffer provide); each
                # tile is 5 bank-sized matmuls + exact max8/max_index
                def tile_group(iv):
                    src = hstack[:, bass.ds(iv * 128 * tk, 128 * tk)]
                    if cpe == "scalar":
                        nc.scalar.activation(wstage[:, :], src, AF.Copy)
                    else:
                        nc.vector.tensor_copy(wstage[:, :], src)
                    for r in range(tk):
                        lhs = wstage[:, r * 128:(r + 1) * 128]
                        for s0 in range(0, NPROJ, 512):
                            sw = min(512, NPROJ - s0)
                            nc.tensor.matmul(ps[:, s0:s0 + sw], lhsT=lhs,
                                             rhs=t_tstk[:, s0:s0 + sw],
                                             start=True, stop=True)
                        nc.vector.max(out=m8, in_=ps[:, 0:NPROJ])
                        nc.vector.max_index(
                            out=outbuf[:, bass.ds(iv * 8 * tk + r * 8, 8)],
                            in_max=m8, in_values=ps[:, 0:NPROJ])
                if hwloop:
                    with tc.For_i(0, NTILES // tk, 1, staggered_reset=sr) as iv:
                        tile_group(iv)
                else:
                    for k in range(NTILES // tk):
                        tile_group(k)

            nc.sync.dma_start(out=out_ext.ap(), in_=outbuf)
    nc.compile()
    return nc


def _host_prep(inputs):
    worker_ids = np.asarray(inputs["worker_ids"]).astype(np.int64)
    project_ids = np.asarray(inputs["project_ids"]).astype(np.int64)
    worker_emb = np.asarray(inputs["worker_emb"], dtype=np.float32)
    project_emb = np.asarray(inputs["project_emb"], dtype=np.float32)
    W1 = np.asarray(inputs["W1"], dtype=np.float32)
    b1 = np.asarray(inputs["b1"], dtype=np.float32)
    W2 = np.asarray(inputs["W2"], dtype=np.float32)
    b2 = np.asarray(inputs["b2"], dtype=np.float32)

    table = project_emb[1:]
    G = (table @ W2).astype(np.float32)
    c = (table @ b2).astype(np.float32)
    G1 = _bf16(G)
    G2 = _bf16(G - G1.astype(np.float32))
    c1 = _bf16(c)
    c2 = _bf16(c - c1.astype(np.float32))
    tstk = np.zeros((122, NPROJ), dtype=ml_dtypes.bfloat16)
    tstk[0:40] = G1.T
    tstk[40:64] = G2.T[0:24]
    tstk[64:104] = G1.T
    tstk[104:120] = G2.T[24:40]
    tstk[120] = c1
    tstk[121] = c2

    # combined gather table: Q7 core c (16 partitions) serves one
    # (table, batch-half): rows 0:10 worker, 16:26 project, 32:42 worker,
    # 48:58 project
    xtab = np.zeros((64, NPTAB), dtype=np.float32)
    xtab[0:EMB, 0:NW] = worker_emb.T
    xtab[16:16 + EMB] = project_emb.T
    xtab[32:32 + EMB, 0:NW] = worker_emb.T
    xtab[48:48 + EMB] = project_emb.T

    # block lhsT [64, 80]: out rows 0:40 = h(half 0), 40:80 = h(half 1)
    w1s = np.zeros((64, 80), dtype=np.float32)
    w1s[0:EMB, 0:HID] = W1[:, 0:EMB].T
    w1s[16:16 + EMB, 0:HID] = W1[:, EMB:2 * EMB].T
    w1s[32:32 + EMB, HID:80] = W1[:, 0:EMB].T
    w1s[48:48 + EMB, HID:80] = W1[:, EMB:2 * EMB].T
    b1e = np.concatenate([b1, b1]).reshape(80, 1).astype(np.float32)

    def wrap16(ids):
        # num_idxs wrapped across a core's 16 partitions: idx i lives at
        # [i % 16, i // 16]
        return ids.astype(np.int16).reshape(-1, 16).T.copy()

    shared = {"xtab": xtab, "w1s": w1s, "b1e": b1e, "tstk": tstk}
    in_maps = []
    for core in range(NCORES):
        sl0 = slice(core * BC, core * BC + HALF)
        sl1 = slice(core * BC + HALF, (core + 1) * BC)
        xi = np.zeros((64, HALF // 16), dtype=np.int16)
        xi[0:16] = wrap16(worker_ids[sl0])
        xi[16:32] = wrap16(project_ids[sl0])
        xi[32:48] = wrap16(worker_ids[sl1])
        xi[48:64] = wrap16(project_ids[sl1])
        m = dict(shared)
        m["xidx"] = xi
        in_maps.append(m)
    return in_maps


def _decode(results):
    idx = np.zeros((B,), dtype=np.int64)
    for core in range(NCORES):
        o = results[core]["out"]          # [128, 8*NTILES] uint32
        for t in range(NTILES):
            rows = slice(core * BC + t * 128, core * BC + (t + 1) * 128)
            idx[rows] = o[:, 8 * t]
    return (idx + 1).astype(np.int32).reshape(B, 1)


def kernel(**inputs):
    from concourse.bass_utils import run_bass_kernel_spmd
    in_maps = _host_prep(inputs)
    if "nc1" not in _cache:
        _cache["nc1"] = _build(L=1)
    res = run_bass_kernel_spmd(_cache["nc1"], in_maps, core_ids=list(range(NCORES)))
    return _decode(res.results)



# revision 2
# speedup vs baseline: 3.6927x; 3.6927x over previous
"""TRN2 Bass kernel for nn_Actor (retrieval_knn).

Data-parallel over batch across 8 NeuronCores (8192 rows/core).

The execution environment pays a large fixed cost per *instruction*
(measured ~10us/instruction regardless of operand size or engine
parallelism), so this kernel minimizes dynamic instruction count:

- ONE ap_gather builds the MLP input directly: channels grouped so each
  Q7 core (16 channels sharing an index stream) serves one
  (table, batch-half) pair -> x64 [64, 4096] with worker/project
  features of both batch halves stacked on partitions.
- MLP layer 1 as 8 matmuls with a [64, 80] block lhsT computing two
  batch-halves at once ([80, 512] PSUM outputs), 2 fused ReLUs.
- h split into bf16 pair (h1, h2 = h - h1) with 2 elementwise ops, then
  6 DMAs assemble the 122-row stacked lhs (W2 absorbed into the table:
  scores = h1G1 + h1G2 + h2G1 + c1 + c2, fp32-grade accuracy).
- Scores + argmax: per 128-row tile, 5 matmuls (PSUM-bank-sized) with
  lhsT taken directly from the stacked buffer (no staging copy), then
  DVE max8 + max_index on the fp32 PSUM scores: exact argmax.
"""
import sys
sys.path.insert(0, '/opt/trn_rl_repo')
import numpy as np
import ml_dtypes

B = 65536
NCORES = 8
BC = B // NCORES            # 8192
NW, NPTAB, EMB = 1807, 2490, 10
NPROJ = NPTAB - 1           # 2489
HID = 40
NTILES = BC // 128          # 64
HALF = BC // 2              # 4096

_cache = {}


def _bf16(x):
    return np.asarray(x, np.float32).astype(ml_dtypes.bfloat16)


def _build(L=1, hwloop=True, tk=1, sr=True, cpe="dma"):
    from concourse import bacc, mybir, bass
    from concourse.tile import TileContext
    import concourse.mybir as mb
    dt = mybir.dt
    AF = mb.ActivationFunctionType
    nc = bacc.Bacc("TRN2", target_bir_lowering=False, debug=False, num_devices=NCORES)

    xidx = nc.dram_tensor("xidx", [64, HALF // 16], dt.int16, kind="ExternalInput")
    xtab = nc.dram_tensor("xtab", [64, NPTAB], dt.float32, kind="ExternalInput")
    w1s = nc.dram_tensor("w1s", [64, 80], dt.float32, kind="ExternalInput")
    b1e = nc.dram_tensor("b1e", [80, 1], dt.float32, kind="ExternalInput")
    tstk = nc.dram_tensor("tstk", [122, NPROJ], dt.bfloat16, kind="ExternalInput")
    out_ext = nc.dram_tensor("out", [128, NTILES * 8], dt.uint32, kind="ExternalOutput")

    with TileContext(nc) as tc:
        with tc.tile_pool(name="const", bufs=1) as cp, \
             tc.tile_pool(name="work", bufs=1) as wp, \
             tc.tile_pool(name="sc", bufs=1, space="PSUM") as scp:
            t_xtab = cp.tile([64, NPTAB], dt.float32)
            t_xidx = cp.tile([64, HALF // 16], dt.int16)
            t_w1s = cp.tile([64, 80], dt.float32)
            t_b1 = cp.tile([80, 1], dt.float32)
            t_tstk = cp.tile([122, NPROJ], dt.bfloat16)
            nc.sync.dma_start(out=t_xtab, in_=xtab.ap())
            nc.sync.dma_start(out=t_xidx, in_=xidx.ap())
            nc.sync.dma_start(out=t_w1s, in_=w1s.ap())
            nc.sync.dma_start(out=t_b1, in_=b1e.ap())
            nc.sync.dma_start(out=t_tstk, in_=tstk.ap())

            x64 = wp.tile([64, HALF], dt.float32)
            h80 = wp.tile([80, BC // 2], dt.float32)
            hs1 = wp.tile([80, BC // 2], dt.bfloat16)
            hs2 = wp.tile([80, BC // 2], dt.bfloat16)
            hstack = wp.tile([122, BC], dt.bfloat16)
            onesrow = wp.tile([2, BC], dt.bfloat16)
            outbuf = wp.tile([128, NTILES * 8], dt.uint32)
            wstage = wp.tile([122, 128 * tk], dt.bfloat16)
            m8 = wp.tile([128, 8], dt.float32)
            nc.vector.memset(onesrow, 1.0)
            nc.sync.dma_start(out=hstack[120:122, :], in_=onesrow)
            ps = scp.tile([128, NPROJ], dt.float32)

            for _ in range(L):
                nc.gpsimd.ap_gather(out_ap=x64, in_ap=t_xtab, idxs_ap=t_xidx,
                                    channels=64, num_elems=NPTAB, d=1,
                                    num_idxs=HALF)
                # MLP layer 1: [64,80] block lhsT -> [80,512] PSUM chunks,
                # two fused 4-bank ReLUs (fully unrolled: only 10 instructions)
                for grp in range(2):
                    for k in range(4):
                        c = grp * 4 + k
                        nc.tensor.matmul(ps[0:80, k * 512:(k + 1) * 512],
                                         lhsT=t_w1s[:, :],
                                         rhs=x64[:, c * 512:(c + 1) * 512],
                                         start=True, stop=True)
                    nc.scalar.activation(h80[:, grp * 2048:(grp + 1) * 2048],
                                         ps[0:80, 0:2048],
                                         mb.ActivationFunctionType.Relu,
                                         bias=t_b1)
                # bf16 split and 122-row lhs assembly
                nc.vector.tensor_copy(hs1[:, :], h80[:, :])
                nc.vector.tensor_sub(hs2[:, :], h80[:, :], hs1[:, :])
                nc.sync.dma_start(out=hstack[0:40, 0:HALF], in_=hs1[0:40, :])
                nc.scalar.dma_start(out=hstack[0:40, HALF:BC], in_=hs1[40:80, :])
                nc.sync.dma_start(out=hstack[64:104, 0:HALF], in_=hs2[0:40, :])
                nc.scalar.dma_start(out=hstack[64:104, HALF:BC], in_=hs2[40:80, :])
                nc.sync.dma_start(out=hstack[40:64, :], in_=hstack[0:24, :])
                nc.scalar.dma_start(out=hstack[104:120, :], in_=hstack[24:40, :])

                # scores + argmax: tk tiles per loop iteration share ONE
                # wstage group copy (ldweights needs static lhsT addresses,
                # which the static slices of the group buffer provide); each
                # tile is 5 bank-sized matmuls + exact max8/max_index
                def tile_group(iv):
                    src = hstack[:, bass.ds(iv * 128 * tk, 128 * tk)]
                    if cpe == "scalar":
                        nc.scalar.activation(wstage[:, :], src, AF.Copy)
                    elif cpe == "dma":
                        nc.sync.dma_start(out=wstage[:, :], in_=src)
                    else:
                        nc.vector.tensor_copy(wstage[:, :], src)
                    for r in range(tk):
                        lhs = wstage[:, r * 128:(r + 1) * 128]
                        for s0 in range(0, NPROJ, 512):
                            sw = min(512, NPROJ - s0)
                            nc.tensor.matmul(ps[:, s0:s0 + sw], lhsT=lhs,
                                             rhs=t_tstk[:, s0:s0 + sw],
                                             start=True, stop=True)
                        nc.vector.max(out=m8, in_=ps[:, 0:NPROJ])
                        nc.vector.max_index(
                            out=outbuf[:, bass.ds(iv * 8 * tk + r * 8, 8)],
                            in_max=m8, in_values=ps[:, 0:NPROJ])
                if hwloop:
                    with tc.For_i(0, NTILES // tk, 1, staggered_reset=sr) as iv:
                        tile_group(iv)
                else:
                    for k in range(NTILES // tk):
                        tile_group(k)

            nc.sync.dma_start(out=out_ext.ap(), in_=outbuf)
    nc.compile()
    return nc


def _host_prep(inputs):
    worker_ids = np.asarray(inputs["worker_ids"]).astype(np.int64)
    project_ids = np.asarray(inputs["project_ids"]).astype(np.int64)
    worker_emb = np.asarray(inputs["worker_emb"], dtype=np.float32)
    project_emb = np.asarray(inputs["project_emb"], dtype=np.float32)
    W1 = np.asarray(inputs["W1"], dtype=np.float32)
    b1 = np.asarray(inputs["b1"], dtype=np.float32)
    W2 = np.asarray(inputs["W2"], dtype=np.float32)
    b2 = np.asarray(inputs["b2"], dtype=np.float32)

    table = project_emb[1:]
    G = (table @ W2).astype(np.float32)
    c = (table @ b2).astype(np.float32)
    G1 = _bf16(G)
    G2 = _bf16(G - G1.astype(np.float32))
    c1 = _bf16(c)
    c2 = _bf16(c - c1.astype(np.float32))
    tstk = np.zeros((122, NPROJ), dtype=ml_dtypes.bfloat16)
    tstk[0:40] = G1.T
    tstk[40:64] = G2.T[0:24]
    tstk[64:104] = G1.T
    tstk[104:120] = G2.T[24:40]
    tstk[120] = c1
    tstk[121] = c2

    # combined gather table: Q7 core c (16 partitions) serves one
    # (table, batch-half): rows 0:10 worker, 16:26 project, 32:42 worker,
    # 48:58 project
    xtab = np.zeros((64, NPTAB), dtype=np.float32)
    xtab[0:EMB, 0:NW] = worker_emb.T
    xtab[16:16 + EMB] = project_emb.T
    xtab[32:32 + EMB, 0:NW] = worker_emb.T
    xtab[48:48 + EMB] = project_emb.T

    # block lhsT [64, 80]: out rows 0:40 = h(half 0), 40:80 = h(half 1)
    w1s = np.zeros((64, 80), dtype=np.float32)
    w1s[0:EMB, 0:HID] = W1[:, 0:EMB].T
    w1s[16:16 + EMB, 0:HID] = W1[:, EMB:2 * EMB].T
    w1s[32:32 + EMB, HID:80] = W1[:, 0:EMB].T
    w1s[48:48 + EMB, HID:80] = W1[:, EMB:2 * EMB].T
    b1e = np.concatenate([b1, b1]).reshape(80, 1).astype(np.float32)

    def wrap16(ids):
        # num_idxs wrapped across a core's 16 partitions: idx i lives at
        # [i % 16, i // 16]
        return ids.astype(np.int16).reshape(-1, 16).T.copy()

    shared = {"xtab": xtab, "w1s": w1s, "b1e": b1e, "tstk": tstk}
    in_maps = []
    for core in range(NCORES):
        sl0 = slice(core * BC, core * BC + HALF)
        sl1 = slice(core * BC + HALF, (core + 1) * BC)
        xi = np.zeros((64, HALF // 16), dtype=np.int16)
        xi[0:16] = wrap16(worker_ids[sl0])
        xi[16:32] = wrap16(project_ids[sl0])
        xi[32:48] = wrap16(worker_ids[sl1])
        xi[48:64] = wrap16(project_ids[sl1])
        m = dict(shared)
        m["xidx"] = xi
        in_maps.append(m)
    return in_maps


def _decode(results):
    idx = np.zeros((B,), dtype=np.int64)
    for core in range(NCORES):
        o = results[core]["out"]          # [128, 8*NTILES] uint32
        for t in range(NTILES):
            rows = slice(core * BC + t * 128, core * BC + (t + 1) * 128)
            idx[rows] = o[:, 8 * t]
    return (idx + 1).astype(np.int32).reshape(B, 1)


def kernel(**inputs):
    from concourse.bass_utils import run_bass_kernel_spmd
    in_maps = _host_prep(inputs)
    if "nc1" not in _cache:
        _cache["nc1"] = _build(L=1)
    res = run_bass_kernel_spmd(_cache["nc1"], in_maps, core_ids=list(range(NCORES)))
    return _decode(res.results)



# revision 3
# speedup vs baseline: 48.3882x; 13.1039x over previous
"""TRN2 Bass kernel for nn_Actor (retrieval_knn).

Data-parallel over batch across 8 NeuronCores (8192 rows/core).

The execution environment pays a large fixed cost per *instruction*
(measured ~10us/instruction regardless of operand size or engine
parallelism), so this kernel minimizes dynamic instruction count:

- ONE ap_gather builds the MLP input directly: channels grouped so each
  Q7 core (16 channels sharing an index stream) serves one
  (table, batch-half) pair -> x64 [64, 4096] with worker/project
  features of both batch halves stacked on partitions.
- MLP layer 1 as 8 matmuls with a [64, 80] block lhsT computing two
  batch-halves at once ([80, 512] PSUM outputs), 2 fused ReLUs.
- h split into bf16 pair (h1, h2 = h - h1) with 2 elementwise ops, then
  6 DMAs assemble the 122-row stacked lhs (W2 absorbed into the table:
  scores = h1G1 + h1G2 + h2G1 + c1 + c2, fp32-grade accuracy).
- Scores + argmax: per 128-row tile, 5 matmuls (PSUM-bank-sized) with
  lhsT taken directly from the stacked buffer (no staging copy), then
  DVE max8 + max_index on the fp32 PSUM scores: exact argmax.
"""
import sys
sys.path.insert(0, '/opt/trn_rl_repo')
import numpy as np
import ml_dtypes

B = 65536
NCORES = 8
BC = B // NCORES            # 8192
NW, NPTAB, EMB = 1807, 2490, 10
NPROJ = NPTAB - 1           # 2489
HID = 40
NTILES = BC // 128          # 64
HALF = BC // 2              # 4096

_cache = {}


def _bf16(x):
    return np.asarray(x, np.float32).astype(ml_dtypes.bfloat16)


def _build(L=1, hwloop=True, tk=1, sr=True, cpe="dma"):
    from concourse import bacc, mybir, bass
    from concourse.tile import TileContext
    import concourse.mybir as mb
    dt = mybir.dt
    AF = mb.ActivationFunctionType
    nc = bacc.Bacc("TRN2", target_bir_lowering=False, debug=False, num_devices=NCORES)

    xidx = nc.dram_tensor("xidx", [64, HALF // 16], dt.int16, kind="ExternalInput")
    xtab = nc.dram_tensor("xtab", [64, NPTAB], dt.float32, kind="ExternalInput")
    w1s = nc.dram_tensor("w1s", [64, 80], dt.float32, kind="ExternalInput")
    b1e = nc.dram_tensor("b1e", [80, 1], dt.float32, kind="ExternalInput")
    tstk = nc.dram_tensor("tstk", [122, NPROJ], dt.bfloat16, kind="ExternalInput")
    out_ext = nc.dram_tensor("out", [128, NTILES * 8], dt.uint32, kind="ExternalOutput")

    with TileContext(nc) as tc:
        with tc.tile_pool(name="const", bufs=1) as cp, \
             tc.tile_pool(name="work", bufs=1) as wp, \
             tc.tile_pool(name="sc", bufs=1, space="PSUM") as scp:
            t_xtab = cp.tile([64, NPTAB], dt.float32)
            t_xidx = cp.tile([64, HALF // 16], dt.int16)
            t_w1s = cp.tile([64, 80], dt.float32)
            t_b1 = cp.tile([80, 1], dt.float32)
            t_tstk = cp.tile([122, NPROJ], dt.bfloat16)
            nc.sync.dma_start(out=t_xtab, in_=xtab.ap())
            nc.sync.dma_start(out=t_xidx, in_=xidx.ap())
            nc.sync.dma_start(out=t_w1s, in_=w1s.ap())
            nc.sync.dma_start(out=t_b1, in_=b1e.ap())
            nc.sync.dma_start(out=t_tstk, in_=tstk.ap())

            x64 = wp.tile([64, HALF], dt.float32)
            h80 = wp.tile([80, BC // 2], dt.float32)
            hs1 = wp.tile([80, BC // 2], dt.bfloat16)
            hs2 = wp.tile([80, BC // 2], dt.bfloat16)
            hstack = wp.tile([122, BC], dt.bfloat16)
            onesrow = wp.tile([2, BC], dt.bfloat16)
            outbuf = wp.tile([128, NTILES * 8], dt.uint32)
            wstage = wp.tile([122, 128 * tk], dt.bfloat16)
            m8 = wp.tile([128, 8], dt.float32)
            nc.vector.memset(onesrow, 1.0)
            nc.sync.dma_start(out=hstack[120:122, :], in_=onesrow)
            ps = scp.tile([128, NPROJ], dt.float32)

            from contextlib import nullcontext
            lctx = tc.For_i(0, L, 1, staggered_reset=True) if L > 1 \
                else nullcontext()
            with lctx:
                nc.gpsimd.ap_gather(out_ap=x64, in_ap=t_xtab, idxs_ap=t_xidx,
                                    channels=64, num_elems=NPTAB, d=1,
                                    num_idxs=HALF)
                # MLP layer 1: [64,80] block lhsT -> [80,512] PSUM chunks,
                # two fused 4-bank ReLUs (fully unrolled: only 10 instructions)
                for grp in range(2):
                    for k in range(4):
                        c = grp * 4 + k
                        nc.tensor.matmul(ps[0:80, k * 512:(k + 1) * 512],
                                         lhsT=t_w1s[:, :],
                                         rhs=x64[:, c * 512:(c + 1) * 512],
                                         start=True, stop=True)
                    nc.scalar.activation(h80[:, grp * 2048:(grp + 1) * 2048],
                                         ps[0:80, 0:2048],
                                         mb.ActivationFunctionType.Relu,
                                         bias=t_b1)
                # bf16 split and 122-row lhs assembly
                nc.vector.tensor_copy(hs1[:, :], h80[:, :])
                nc.vector.tensor_sub(hs2[:, :], h80[:, :], hs1[:, :])
                nc.sync.dma_start(out=hstack[0:40, 0:HALF], in_=hs1[0:40, :])
                nc.scalar.dma_start(out=hstack[0:40, HALF:BC], in_=hs1[40:80, :])
                nc.sync.dma_start(out=hstack[64:104, 0:HALF], in_=hs2[0:40, :])
                nc.scalar.dma_start(out=hstack[64:104, HALF:BC], in_=hs2[40:80, :])
                nc.sync.dma_start(out=hstack[40:64, :], in_=hstack[0:24, :])
                nc.scalar.dma_start(out=hstack[104:120, :], in_=hstack[24:40, :])

                # scores + argmax: tk tiles per loop iteration share ONE
                # wstage group copy (ldweights needs static lhsT addresses,
                # which the static slices of the group buffer provide); each
                # tile is 5 bank-sized matmuls + exact max8/max_index
                def tile_group(iv):
                    src = hstack[:, bass.ds(iv * 128 * tk, 128 * tk)]
                    if cpe == "scalar":
                        nc.scalar.activation(wstage[:, :], src, AF.Copy)
                    elif cpe == "dma":
                        nc.sync.dma_start(out=wstage[:, :], in_=src)
                    else:
                        nc.vector.tensor_copy(wstage[:, :], src)
                    for r in range(tk):
                        lhs = wstage[:, r * 128:(r + 1) * 128]
                        for s0 in range(0, NPROJ, 512):
                            sw = min(512, NPROJ - s0)
                            nc.tensor.matmul(ps[:, s0:s0 + sw], lhsT=lhs,
                                             rhs=t_tstk[:, s0:s0 + sw],
                                             start=True, stop=True)
                        nc.vector.max(out=m8, in_=ps[:, 0:NPROJ])
                        nc.vector.max_index(
                            out=outbuf[:, bass.ds(iv * 8 * tk + r * 8, 8)],
                            in_max=m8, in_values=ps[:, 0:NPROJ])
                if hwloop:
                    with tc.For_i(0, NTILES // tk, 1, staggered_reset=sr) as iv:
                        tile_group(iv)
                else:
                    for k in range(NTILES // tk):
                        tile_group(k)

            nc.sync.dma_start(out=out_ext.ap(), in_=outbuf)
    nc.compile()
    return nc


def _host_prep(inputs):
    worker_ids = np.asarray(inputs["worker_ids"]).astype(np.int64)
    project_ids = np.asarray(inputs["project_ids"]).astype(np.int64)
    worker_emb = np.asarray(inputs["worker_emb"], dtype=np.float32)
    project_emb = np.asarray(inputs["project_emb"], dtype=np.float32)
    W1 = np.asarray(inputs["W1"], dtype=np.float32)
    b1 = np.asarray(inputs["b1"], dtype=np.float32)
    W2 = np.asarray(inputs["W2"], dtype=np.float32)
    b2 = np.asarray(inputs["b2"], dtype=np.float32)

    table = project_emb[1:]
    G = (table @ W2).astype(np.float32)
    c = (table @ b2).astype(np.float32)
    G1 = _bf16(G)
    G2 = _bf16(G - G1.astype(np.float32))
    c1 = _bf16(c)
    c2 = _bf16(c - c1.astype(np.float32))
    tstk = np.zeros((122, NPROJ), dtype=ml_dtypes.bfloat16)
    tstk[0:40] = G1.T
    tstk[40:64] = G2.T[0:24]
    tstk[64:104] = G1.T
    tstk[104:120] = G2.T[24:40]
    tstk[120] = c1
    tstk[121] = c2

    # combined gather table: Q7 core c (16 partitions) serves one
    # (table, batch-half): rows 0:10 worker, 16:26 project, 32:42 worker,
    # 48:58 project
    xtab = np.zeros((64, NPTAB), dtype=np.float32)
    xtab[0:EMB, 0:NW] = worker_emb.T
    xtab[16:16 + EMB] = project_emb.T
    xtab[32:32 + EMB, 0:NW] = worker_emb.T
    xtab[48:48 + EMB] = project_emb.T

    # block lhsT [64, 80]: out rows 0:40 = h(half 0), 40:80 = h(half 1)
    w1s = np.zeros((64, 80), dtype=np.float32)
    w1s[0:EMB, 0:HID] = W1[:, 0:EMB].T
    w1s[16:16 + EMB, 0:HID] = W1[:, EMB:2 * EMB].T
    w1s[32:32 + EMB, HID:80] = W1[:, 0:EMB].T
    w1s[48:48 + EMB, HID:80] = W1[:, EMB:2 * EMB].T
    b1e = np.concatenate([b1, b1]).reshape(80, 1).astype(np.float32)

    def wrap16(ids):
        # num_idxs wrapped across a core's 16 partitions: idx i lives at
        # [i % 16, i // 16]
        return ids.astype(np.int16).reshape(-1, 16).T.copy()

    shared = {"xtab": xtab, "w1s": w1s, "b1e": b1e, "tstk": tstk}
    in_maps = []
    for core in range(NCORES):
        sl0 = slice(core * BC, core * BC + HALF)
        sl1 = slice(core * BC + HALF, (core + 1) * BC)
        xi = np.zeros((64, HALF // 16), dtype=np.int16)
        xi[0:16] = wrap16(worker_ids[sl0])
        xi[16:32] = wrap16(project_ids[sl0])
        xi[32:48] = wrap16(worker_ids[sl1])
        xi[48:64] = wrap16(project_ids[sl1])
        m = dict(shared)
        m["xidx"] = xi
        in_maps.append(m)
    return in_maps


def _decode(results):
    idx = np.zeros((B,), dtype=np.int64)
    for core in range(NCORES):
        o = results[core]["out"]          # [128, 8*NTILES] uint32
        for t in range(NTILES):
            rows = slice(core * BC + t * 128, core * BC + (t + 1) * 128)
            idx[rows] = o[:, 8 * t]
    return (idx + 1).astype(np.int32).reshape(B, 1)


def kernel(**inputs):
    from concourse.bass_utils import run_bass_kernel_spmd
    in_maps = _host_prep(inputs)
    if "nc1" not in _cache:
        _cache["nc1"] = _build(L=1)
    res = run_bass_kernel_spmd(_cache["nc1"], in_maps, core_ids=list(range(NCORES)))
    return _decode(res.results)

